# revision 42
# baseline (speedup 1.0000x reference)
"""Trainium2 Bass kernel for nn_DecoderWithPositionLayer (8 NeuronCores).

Sharding: 2 groups x 4 cores; group g owns batch g (256 tokens).
Within a group (rank s = core % 4):
  - Every FFN: mm1 hidden-split 4-way (local h = relu(w1_s^T xln + b1_s)),
    mm2 row-parallel over the local hidden slice producing PARTIAL outputs
    for the FULL output dim; partials are combined with a single collective
    AFTER mm2 (no mid-FFN AllGather):
      * block0 q-part / block1 y: token-reordered ReduceScatter(add)
        -> each core gets its 64 query tokens, full feature dim.
      * block0 kv / block2 kv: AllReduce(add) -> full kv on every core.
      * block3: no collective; f32 partials returned, host sums 4 ranks.
  - b1/b2 bias adds ride as rank-1 PE matmuls into the mm PSUM accumulation
    (b2 pre-scaled 0.25 per rank so the collective sum restores it); the
    1/sqrt(dim) score scale is folded into w2/b2 k-columns on the host.
  - Attention split by query tokens (64 q/core). Relative-position bias
    reassociated: bias[q,k] = sum_f qp[q,f] rp[q,k,f],
                  qp[h,q,f] = sum_d q[h,q,d] pos_w[f, h*64+d]
    (pos_b is row-constant in softmax -> dropped). Causal mask rides as an
    extra contraction row of host-transposed rp. Softmax-weight and V
    transposes are PE is_transpose matmuls.
  - All host-side tensors are laid out partition-major so every big DMA
    moves multi-KB contiguous runs per partition.
Activations stay feature-major [feature, token]; matmuls bf16 with f32
PSUM accumulation; LN/softmax math f32.
"""

import contextlib
import numpy as np
import ml_dtypes

import concourse.bass as bass
import concourse.bacc as bacc_mod
import concourse.tile as tile
from concourse import mybir
from concourse.bass_utils import run_bass_kernel_spmd

BF16 = ml_dtypes.bfloat16
FP32 = mybir.dt.float32
BF = mybir.dt.bfloat16

B, S, D, F, HID, H = 2, 256, 1024, 64, 4096, 16
DIM = D // H
G = 4
TLOC = S // G          # 64
HIDL = HID // G        # 1024
P = 128
EPS = 1e-3
NEG = -1e9
OUTD = [3 * D, D, 2 * D, D]
RG = [[0, 1, 2, 3], [4, 5, 6, 7]]

_CACHE = {}


def _pmajor(a):
    """[128*k, N] -> [128, k, N] partition-major contiguous."""
    rows, n = a.shape
    k = rows // P
    return np.ascontiguousarray(a.reshape(k, P, n).transpose(1, 0, 2))


# ------------------------------------------------------------------ host prep
def _prep_in_maps(inp):
    f32 = np.float32
    qT = np.ascontiguousarray(np.transpose(np.asarray(inp["queries"], f32), (0, 2, 1)))
    vT = np.ascontiguousarray(np.transpose(np.asarray(inp["values"], f32), (0, 2, 1)))
    pw = np.asarray(inp["pos_w"], f32)                        # [F, D]
    poswT = np.ascontiguousarray(
        pw.reshape(F, H, DIM).transpose(2, 1, 0)).astype(BF16)  # [d, h, f]

    rp = np.asarray(inp["relative_positions"], f32)
    rpT = np.transpose(rp, (0, 1, 3, 2))                       # [B,S,F,S]
    mask = np.where(np.arange(S)[None, :] <= np.arange(S)[:, None], 0.0, NEG)
    rpT_ext = np.concatenate(
        [rpT, np.broadcast_to(mask[None, :, None, :], (B, S, 1, S))], axis=2
    ).astype(BF16)                                             # [B,S,F+1,S]

    # fold 1/8 score scale into block0/block2 k-columns of w2 (and b2),
    # then permute columns into contiguous [q|k|v] / [k|v] blocks
    kcol0 = np.zeros(OUTD[0], bool)
    for h in range(H):
        kcol0[h * 192 + 64:h * 192 + 128] = True
    kcol2 = np.zeros(OUTD[2], bool)
    for h in range(H):
        kcol2[h * 128:h * 128 + 64] = True
    kcols = {0: kcol0, 2: kcol2}
    perm0 = np.concatenate([
        np.arange(OUTD[0]).reshape(H, 3, DIM)[:, part, :].reshape(-1)
        for part in range(3)])
    perm2 = np.concatenate([
        np.arange(OUTD[2]).reshape(H, 2, DIM)[:, part, :].reshape(-1)
        for part in range(2)])
    perms = {0: perm0, 2: perm2}

    blocks = []
    for i in range(4):
        p = f"b{i}_"
        g = np.asarray(inp[p + "ln_g"], f32)
        be = np.asarray(inp[p + "ln_b"], f32)
        w1 = np.asarray(inp[p + "w1"], f32)
        b1 = np.asarray(inp[p + "b1"], f32)
        w2 = np.asarray(inp[p + "w2"], f32).copy()
        b2 = np.asarray(inp[p + "b2"], f32).copy()
        if i in kcols:
            w2[:, kcols[i]] *= 0.125
            b2[kcols[i]] *= 0.125
            w2 = w2[:, perms[i]]
            b2 = b2[perms[i]]
        blocks.append((g[:, None] * w1, b1 + be @ w1, w2, b2))

    def ln_host(x):
        mu = x.mean(0, keepdims=True)
        r = 1.0 / np.sqrt(x.var(0, keepdims=True) + EPS)
        return ((x - mu) * r)

    qhat = np.stack([ln_host(qT[g]) for g in range(B)])   # [B, D, S]
    vhat = np.stack([ln_host(vT[g]) for g in range(B)])

    in_maps = []
    for c in range(8):
        g, s = c // G, c % G
        # rp for this core's 64 q tokens: [4 slab][65 f][16 q][256 k]
        rp_c = rpT_ext[g, s * TLOC:(s + 1) * TLOC].transpose(1, 0, 2)  # [65,64,256]
        rp_c = np.ascontiguousarray(
            rp_c.reshape(F + 1, 4, 16, S).transpose(1, 0, 2, 3))
        m = {
            "xT": _pmajor(qhat[g]).astype(BF16),
            "vT": _pmajor(vhat[g]).astype(BF16),
            "xTslice": _pmajor(np.ascontiguousarray(
                qT[g][:, s * TLOC:(s + 1) * TLOC])),
            "pos_wT": poswT,
            "rpT": rp_c,
        }
        for i, (w1f, b1f, w2, b2) in enumerate(blocks):
            w1s = np.ascontiguousarray(w1f[:, s * HIDL:(s + 1) * HIDL])
            m[f"w1_{i}"] = _pmajor(w1s).astype(BF16)
            m[f"w1sum_{i}"] = w1s.sum(axis=0, keepdims=True).astype(BF16)
            m[f"b1_{i}"] = np.ascontiguousarray(
                b1f[s * HIDL:(s + 1) * HIDL].reshape(8, P).T).astype(f32)
            m[f"w2_{i}"] = _pmajor(w2[s * HIDL:(s + 1) * HIDL, :]).astype(BF16)
            m[f"b2_{i}"] = np.ascontiguousarray(
                (0.25 * b2).reshape(-1, P).T).astype(f32)
        in_maps.append(m)
    return in_maps


# --------------------------------------------------------------- device build
def _build_nc():
    nc = bacc_mod.Bacc(num_devices=8)
    io = {}
    io["xT"] = nc.declare_dram_parameter("xT", [P, 8, S], BF, False)
    io["vT"] = nc.declare_dram_parameter("vT", [P, 8, S], BF, False)
    io["xTslice"] = nc.declare_dram_parameter("xTslice", [P, 8, TLOC], FP32, False)
    io["pos_wT"] = nc.declare_dram_parameter("pos_wT", [DIM, H, F], BF, False)
    io["rpT"] = nc.declare_dram_parameter("rpT", [4, F + 1, 16, S], BF, False)
    for i in range(4):
        io[f"w1_{i}"] = nc.declare_dram_parameter(f"w1_{i}", [P, 8, HIDL], BF, False)
        io[f"w1sum_{i}"] = nc.declare_dram_parameter(f"w1sum_{i}", [1, HIDL], BF, False)
        io[f"b1_{i}"] = nc.declare_dram_parameter(f"b1_{i}", [P, 8], FP32, False)
        io[f"w2_{i}"] = nc.declare_dram_parameter(f"w2_{i}", [P, 8, OUTD[i]], BF, False)
        io[f"b2_{i}"] = nc.declare_dram_parameter(
            f"b2_{i}", [P, OUTD[i] // P], FP32, False)
    io["x2T_out"] = nc.declare_dram_parameter("x2T_out", [D, TLOC], FP32, True)
    io["o3T_out"] = nc.declare_dram_parameter("o3T_out", [D, S], BF, True)

    with tile.TileContext(nc) as tc:
        _program(nc, tc, io)
    nc.finalize()
    return nc


def _ap(t, offset, pattern):
    tensor = t.tensor if isinstance(t, bass.AP) else t.ap().tensor
    return bass.AP(tensor=tensor, offset=offset, ap=pattern)


def _program(nc, tc, io):
    import os
    KMODE = int(os.environ.get("KMODE", "9"))
    ctx = contextlib.ExitStack()
    sb = ctx.enter_context(tc.tile_pool(name="sb", bufs=2))
    consts = ctx.enter_context(tc.tile_pool(name="consts", bufs=1))
    psum = ctx.enter_context(tc.tile_pool(name="psum", bufs=2, space="PSUM"))
    dram = ctx.enter_context(tc.tile_pool(name="dram", bufs=1, space="DRAM"))

    sync, vec, act, pe, gps = nc.sync, nc.vector, nc.scalar, nc.tensor, nc.gpsimd
    AF = mybir.ActivationFunctionType
    ALU = mybir.AluOpType

    ones_col = consts.tile([P, 1], BF, tag="ones_col")
    vec.memset(ones_col, 1.0)
    ones_colf = consts.tile([P, 1], FP32, tag="ones_colf")
    vec.memset(ones_colf, 1.0)
    ones_row = consts.tile([1, S], BF, tag="ones_row")
    vec.memset(ones_row, 1.0)

    from concourse.masks import make_identity
    ident = consts.tile([P, P], BF, tag="ident")
    make_identity(nc, ident)

    # ---- t0 prefetches (all contiguous partition-major) ----
    poswT = consts.tile([DIM, H, F], BF, tag="poswT")
    sync.dma_start(out=poswT, in_=io["pos_wT"].ap())
    xs_all = consts.tile([P, 8, TLOC], FP32, tag="xslice")
    sync.dma_start(out=xs_all, in_=io["xTslice"].ap())

    # ---------------------------------------------------------------- helpers
    def ln_finish(ps_sum, ps_sq, xt, T, tag):
        mu = sb.tile([1, T], FP32, tag="lnstat", bufs=6)
        vec.tensor_scalar_mul(mu, ps_sum[0:1, :], 1.0 / D)
        m2 = sb.tile([1, T], FP32, tag="lnstat", bufs=6)
        vec.tensor_scalar_mul(m2, ps_sq[0:1, :], 1.0 / D)
        ve = sb.tile([1, T], FP32, tag="lnstat", bufs=6)
        vec.tensor_mul(ve, mu, mu)
        vec.tensor_sub(ve, m2, ve)
        vec.tensor_scalar_add(ve, ve, EPS)
        rinv = sb.tile([1, T], FP32, tag="lnstat", bufs=6)
        vec.reciprocal(rinv, ve)
        r = sb.tile([1, T], FP32, tag="lnstat", bufs=6)
        act.sqrt(r, rinv)
        mr = sb.tile([1, T], FP32, tag="lnstat", bufs=6)
        vec.tensor_mul(mr, mu, r)
        negmur = sb.tile([1, T], BF, tag="negmur", bufs=4)
        vec.tensor_scalar_mul(negmur, mr, -1.0)
        rb = sb.tile([1, T], BF, tag="lnstatb", bufs=2)
        vec.tensor_copy(rb, r)
        ps_b = psum.tile([P, T], FP32, tag="ps_stat", bufs=2)
        pe.matmul(ps_b, ones_row[:, 0:P], rb, start=True, stop=True)
        r_bc = sb.tile([P, T], FP32, tag="r_bc", bufs=2)
        vec.tensor_copy(r_bc, ps_b)
        xsca = sb.tile([P, 8, T], BF, tag=f"xsc_{tag}", bufs=1)
        for k in range(8):
            vec.tensor_mul(xsca[:, k, :], xt[k], r_bc)
        return [xsca[:, k, :] for k in range(8)], negmur, xsca

    def ln_scale(xt, T, tag):
        """xt: list of 8 [128,T] f32 SBUF tiles (feature-major)."""
        ps_sum = psum.tile([P, T], FP32, tag="ps_stat", bufs=2)
        ps_sq = psum.tile([P, T], FP32, tag="ps_stat", bufs=2)
        sqa = sb.tile([P, 8, T], BF, tag="lnsq", bufs=1,
                      padded_shape=[P, 8, TLOC])
        for k in range(8):
            pe.matmul(ps_sum[0:1, :], ones_colf, xt[k],
                      start=(k == 0), stop=(k == 7))
        for k in range(8):
            vec.tensor_mul(sqa[:, k, :], xt[k], xt[k])
            pe.matmul(ps_sq[0:1, :], ones_col, sqa[:, k, :],
                      start=(k == 0), stop=(k == 7))
        return ln_finish(ps_sum, ps_sq, xt, T, tag)

    def load_feature_major(handle, eng=sync):
        xf = sb.tile([P, 8, S], BF, tag="xbf", bufs=2)
        eng.dma_start(out=xf, in_=handle.ap())
        return [xf[:, k, :] for k in range(8)]

    def ffn_weights(i, weng):
        """Prefetch FFN i weights; smalls first so they never queue behind
        the multi-MB w1/w2 transfers (whose WAR stalls block the ring)."""
        od = OUTD[i]
        w1sum = None
        if i in (1, 3):
            w1sum = sb.tile([1, HIDL], BF, tag="w1sum", bufs=2)
            act.dma_start(out=w1sum, in_=io[f"w1sum_{i}"].ap())
        b1 = consts.tile([P, 8], FP32, tag=f"b1_{i}")
        act.dma_start(out=b1, in_=io[f"b1_{i}"].ap())
        b2 = consts.tile([P, od // P], FP32, tag=f"b2_{i}")
        act.dma_start(out=b2, in_=io[f"b2_{i}"].ap())
        w1t = w2r = None
        if weng is not None:
            w1t = sb.tile([P, 8, HIDL], BF, tag="w1", bufs=2)
            weng.dma_start(out=w1t, in_=io[f"w1_{i}"].ap())
            if od > D:
                half = od // 2
                w2a = sb.tile([P, 8, half], BF, tag="w2", bufs=2,
                              padded_shape=[P, 8, 3 * D // 2])
                weng.dma_start(out=w2a, in_=_ap(io[f"w2_{i}"], 0,
                                                [[8 * od, P], [od, 8], [1, half]]))
                w2b = sb.tile([P, 8, half], BF, tag="w2", bufs=2,
                              padded_shape=[P, 8, 3 * D // 2])
                weng.dma_start(out=w2b, in_=_ap(io[f"w2_{i}"], half,
                                                [[8 * od, P], [od, 8], [1, half]]))
                w2r = (w2a, w2b, half)
            else:
                w2a = sb.tile([P, 8, od], BF, tag="w2", bufs=2,
                              padded_shape=[P, 8, 3 * D // 2])
                weng.dma_start(out=w2a, in_=io[f"w2_{i}"].ap())
                w2r = (w2a, None, od)
        return w1t, w2r, w1sum, b1, b2

    def ffn_mm1(W, mv, negmur):
        """mm1 (hidden-split) -> relu -> local h."""
        w1t, w2r, w1sum, b1, b2 = W
        h = sb.tile([P, 8, S], BF, tag="h", bufs=2)
        for m in range(8):
            ps = psum.tile([P, S], FP32, tag="ps_mm", bufs=2)
            for k in range(8):
                pe.matmul(ps, w1t[:, k, m * P:(m + 1) * P], mv[k],
                          start=(k == 0), stop=(k == 7 and negmur is None))
            if negmur is not None:
                pe.matmul(ps, w1sum[:, m * P:(m + 1) * P], negmur,
                          start=False, stop=True)
            vec.tensor_scalar(h[:, m, :], ps, b1[:, m:m + 1], 0.0,
                              op0=ALU.add, op1=ALU.max)
        return h

    def ffn_mm2(i, W, h, emit, morder=None):
        """mm2 row-parallel over local hidden; emit(m, b2, ps) per chunk."""
        w1t, (w2a, w2b, half), w1sum, b1, b2 = W
        for m in (morder or range(OUTD[i] // P)):
            w2t = w2a if m * P < half else w2b
            c0 = m * P - (0 if m * P < half else half)
            ps = psum.tile([P, S], FP32, tag="ps_mm", bufs=2)
            for k in range(8):
                pe.matmul(ps, w2t[:, k, c0:c0 + P], h[:, k, :],
                          start=(k == 0), stop=(k == 7))
            emit(m, b2, ps)

    def ffn(i, W, mv, negmur, emit, morder=None):
        ffn_mm2(i, W, ffn_mm1(W, mv, negmur), emit, morder)

    def load_rp16(sl):
        rp16 = sb.tile([F + 1, 16, S], BF, tag="rp16", bufs=2)
        gps.dma_start(out=rp16, in_=_ap(io["rpT"], sl * (F + 1) * 16 * S,
                                        [[16 * S, F + 1], [1, 16 * S]]))
        return rp16

    # =================== LN0 + block0 (fused qkv) ===================
    # priority load order on the sync ring: xT -> w1_0 -> vT -> w2_0
    xt0 = load_feature_major(io["xT"])
    W0w1 = sb.tile([P, 8, HIDL], BF, tag="w1", bufs=2)
    sync.dma_start(out=W0w1, in_=io["w1_0"].ap())
    xt2 = load_feature_major(io["vT"])
    W0a = sb.tile([P, 8, OUTD[0] // 2], BF, tag="w2", bufs=2,
                  padded_shape=[P, 8, 3 * D // 2])
    sync.dma_start(out=W0a, in_=_ap(io["w2_0"], 0,
                                    [[8 * OUTD[0], P], [OUTD[0], 8],
                                     [1, OUTD[0] // 2]]))
    W0b = sb.tile([P, 8, OUTD[0] // 2], BF, tag="w2", bufs=2,
                  padded_shape=[P, 8, 3 * D // 2])
    sync.dma_start(out=W0b, in_=_ap(io["w2_0"], OUTD[0] // 2,
                                    [[8 * OUTD[0], P], [OUTD[0], 8],
                                     [1, OUTD[0] // 2]]))
    W0s = ffn_weights(0, None)
    W0 = (W0w1, (W0a, W0b, OUTD[0] // 2)) + W0s[2:]
    with tc.tile_wait_until(0.018):
        W2 = ffn_weights(2, act)
    xsc0, negmur0 = xt0, None
    xsc2, negmur2 = xt2, None
    # token-reordered partial q -> ReduceScatter; kv partials -> AllReduce
    cc_q0 = dram.tile([G, D, TLOC], BF, tag="cc_q0")
    qg = dram.tile([D, TLOC], BF, tag="qg")
    cc_kv0 = dram.tile([2 * D, S], BF, tag="cc_kv0")
    kvg0 = dram.tile([2 * D, S], BF, tag="kvg0")

    def emit_qkv(m, b2t, ps, cc_q, cc_kv, nq):
        o = sb.tile([P, S], BF, tag="qkvband", bufs=4)
        if m % 2:
            vec.tensor_scalar_add(o, ps, b2t[:, m:m + 1])
        else:
            act.add(o, ps, b2t[:, m:m + 1])
        if m < nq:
            sync.dma_start(
                out=_ap(cc_q, m * P * TLOC,
                        [[TLOC, P], [D * TLOC, G], [1, TLOC]]),
                in_=o)
        else:
            row = (m - nq) * P
            gps.dma_start(out=cc_kv[row:row + P, :], in_=o)

    def emit0(m, b2t, ps):
        emit_qkv(m, b2t, ps, cc_q0, cc_kv0, 8)

    # q chunks (m<8) emit first so the ReduceScatter starts early
    rp16s = [load_rp16(0), load_rp16(1)]
    ffn(0, W0, xsc0, negmur0, emit0)
    gps.collective_compute("ReduceScatter", ALU.add, replica_groups=RG,
                           ins=[cc_q0.opt()], outs=[qg.opt()])
    gps.collective_compute("AllReduce", ALU.add, replica_groups=RG,
                           ins=[cc_kv0[0:D, :].opt()], outs=[kvg0[0:D, :].opt()])
    gps.collective_compute("AllReduce", ALU.add, replica_groups=RG,
                           ins=[cc_kv0[D:2 * D, :].opt()],
                           outs=[kvg0[D:2 * D, :].opt()])
    DBG = int(os.environ.get("DBG", "0"))
    if DBG == 1:
        sync.dma_start(out=io["o3T_out"].ap(), in_=_ap(kvg0, 0, [[S, D], [1, S]]))
        ctx.close()
        return
    if DBG == 2:
        sync.dma_start(out=io["o3T_out"].ap()[0:D, 0:TLOC], in_=qg)
        ctx.close()
        return

    if KMODE < 2:
        ctx.close()
        return

    # =================== block2 mm1 ===================
    cc_kv2 = dram.tile([2 * D, S], BF, tag="cc_kv2")
    kvg2 = dram.tile([2 * D, S], BF, tag="kvg2")
    h2 = ffn_mm1(W2, xsc2, negmur2)

    def emit2(m, b2t, ps):
        emit_qkv(m, b2t, ps, None, cc_kv2, 0)

    ffn_mm2(2, W2, h2, emit2)
    gps.collective_compute("AllReduce", ALU.add, replica_groups=RG,
                           ins=[cc_kv2.opt()], outs=[kvg2.opt()])
    kv2_sb = sb.tile([DIM, H, 2, S], BF, tag="kv2", bufs=1)
    gps.dma_start(out=kv2_sb[:, :, 0, :],
                  in_=_ap(kvg2, 0, [[S, DIM], [DIM * S, H], [1, S]]))
    gps.dma_start(out=kv2_sb[:, :, 1, :],
                  in_=_ap(kvg2, D * S, [[S, DIM], [DIM * S, H], [1, S]]))

    if KMODE < 3:
        ctx.close()
        return

    # =================== qp + relative-position bias ===================
    # qh: [64(d), 16(h), 64(q)]
    qh_all = sb.tile([DIM, H, TLOC], BF, tag="qh", bufs=1)
    sync.dma_start(out=qh_all, in_=_ap(qg, 0,
                                       [[TLOC, DIM], [DIM * TLOC, H], [1, TLOC]]))
    kv0_sb = sb.tile([DIM, H, 2, S], BF, tag="kv0", bufs=1)
    sync.dma_start(out=kv0_sb[:, :, 0, :],
                   in_=_ap(kvg0, 0, [[S, DIM], [DIM * S, H], [1, S]]))
    sync.dma_start(out=kv0_sb[:, :, 1, :],
                   in_=_ap(kvg0, D * S, [[S, DIM], [DIM * S, H], [1, S]]))
    qp_ext = sb.tile([F + 1, H, TLOC], BF, tag="qp_ext", bufs=1)
    vec.memset(qp_ext[F:F + 1, :, :], 1.0)
    for h in range(H):
        qp_ps = psum.tile([F, TLOC], FP32, tag="ps_attn", bufs=2)
        pe.matmul(qp_ps, poswT[:, h, :], qh_all[:, h, :], start=True, stop=True)
        (act.copy if h % 2 else vec.tensor_copy)(qp_ext[0:F, h, :], qp_ps)
    # bias per q-group of 4: bp rows qi*32+h, cols k; batched DRAM round trip.
    # rp streamed per q-octet slab: [65(f), 8(q), 256(k)] contiguous
    bias_d = dram.tile([TLOC // 4, P, S], BF, tag="bias_d")

    def bias_half(sls):
        for sl in sls:
            rp16 = rp16s[sl]
            for g4 in range(4):
                g = sl * 4 + g4
                bp = psum.tile([P, S], FP32, tag="ps_attn", bufs=2)
                for qi in range(4):
                    q = g * 4 + qi
                    pe.matmul(bp[qi * 32:qi * 32 + H, :], qp_ext[:, :, q],
                              rp16[:, q - sl * 16, :], start=True, stop=True,
                              tile_position=(0, qi * 32))
                bsb = sb.tile([P, S], BF, tag="bsb", bufs=2)
                if g % 2:
                    vec.tensor_copy(bsb, bp)
                else:
                    act.copy(bsb, bp)
                act.dma_start(out=bias_d[g], in_=bsb)

    bias_half([0, 1])
    rp16s.append(load_rp16(2))
    rp16s.append(load_rp16(3))
    bias_half([2, 3])

    if DBG == 3:
        sync.dma_start(out=io["o3T_out"].ap()[0:P * 4, :],
                       in_=_ap(bias_d.tensor, 0, [[S, P * 4], [1, S]]))
        ctx.close()
        return


    if KMODE < 4:
        ctx.close()
        return

    # =================== attention machinery ===================
    def make_vT(kv_sb, tag):
        # vT: [128(k-token), 16(h), 2(kc), 64(d)] via PE transposes
        vt = sb.tile([P, H, 2, DIM], BF, tag=tag, bufs=1)
        for h in range(H):
            for kc in range(2):
                tr = psum.tile([P, P], BF, tag="ps_tr", bufs=2)
                pe.matmul(tr[:, 0:DIM], kv_sb[:, h, 1, kc * P:(kc + 1) * P],
                          ident[0:DIM, 0:DIM], is_transpose=True)
                eng = act if (h % 2) else vec
                (eng.copy if eng is act else eng.tensor_copy)(
                    vt[:, h, kc, :], tr[:, 0:DIM])
        return vt

    def softmax_av(s_src, vt, consume):
        """s_src(j) -> f32 scores [128,S] for head pair j (psum or sbuf).
        vt: [128, H, 2, DIM]. consume(j, a_ps) eats psum [128(2h*64d), TLOC]."""
        for j in range(H // 2):
            s_t = s_src(j)
            e = sb.tile([P, S], BF, tag="e", bufs=3)
            z = sb.tile([P, 1], FP32, tag="z", bufs=4)
            act.activation(e, s_t, AF.Exp, accum_out=z)
            rz = sb.tile([P, 1], FP32, tag="rz", bufs=4)
            vec.reciprocal(rz, z)
            en = sb.tile([P, S], BF, tag="en", bufs=3)
            vec.tensor_scalar_mul(en, e, rz)
            wt = sb.tile([P, 2, P], BF, tag="wt", bufs=3)
            for kc in range(2):
                tr = psum.tile([P, P], BF, tag="ps_tr", bufs=2)
                pe.matmul(tr, en[:, kc * P:(kc + 1) * P], ident,
                          is_transpose=True)
                eng = act if kc else vec
                (eng.copy if eng is act else eng.tensor_copy)(wt[:, kc, :], tr)
            a_ps = psum.tile([P, TLOC], FP32, tag="ps_mm", bufs=2,
                             padded_shape=[P, S])
            for kc in range(2):
                for hh in range(2):
                    pe.matmul(a_ps[hh * DIM:(hh + 1) * DIM, :],
                              vt[:, 2 * j + hh, kc, :], wt[:, kc, hh * 64:(hh + 1) * 64],
                              start=(kc == 0), stop=(kc == 1))
            consume(j, a_ps)

    # =================== self-attention ===================
    vt0 = make_vT(kv0_sb, "vt0")

    bps = []
    for j in range(H // 2):
        bp = sb.tile([P, S], BF, tag="bp", bufs=8)
        sync.dma_start(
            out=bp, in_=_ap(bias_d.tensor, 2 * j * S,
                            [[S, 2], [P * S, TLOC // 4], [32 * S, 4], [1, S]]))
        bps.append(bp)

    def s_src0(j):
        s_ps = psum.tile([P, S], FP32, tag="ps_attn", bufs=2)
        for hh in range(2):
            h = 2 * j + hh
            pe.matmul(s_ps[hh * 64:(hh + 1) * 64, :], qh_all[:, h, :],
                      kv0_sb[:, h, 0, :], start=True, stop=True)
        vec.tensor_add(s_ps, s_ps, bps[j])
        return s_ps

    # x1 = queries_slice + a0, built pair by pair; LN1 stats ride along
    x1a = sb.tile([P, 8, TLOC], FP32, tag="x1", bufs=1)
    ps_sum1 = psum.tile([P, TLOC], FP32, tag="ps_stat", bufs=2)
    ps_sq1 = psum.tile([P, TLOC], FP32, tag="ps_stat", bufs=2)
    sq1 = sb.tile([P, 8, TLOC], BF, tag="lnsq", bufs=1)

    def consume0(j, a_ps):
        vec.tensor_add(x1a[:, j, :], xs_all[:, j, :], a_ps)
        pe.matmul(ps_sum1[0:1, :], ones_colf, x1a[:, j, :],
                  start=(j == 0), stop=(j == 7))
        vec.tensor_mul(sq1[:, j, :], x1a[:, j, :], x1a[:, j, :])
        pe.matmul(ps_sq1[0:1, :], ones_col, sq1[:, j, :],
                  start=(j == 0), stop=(j == 7))

    softmax_av(s_src0, vt0, consume0)
    W1 = ffn_weights(1, act)
    W3 = ffn_weights(3, act)

    if KMODE < 5:
        ctx.close()
        return

    # LN1 -> AllGather (with negmur row)
    x1t = [x1a[:, k, :] for k in range(8)]
    xsc1, negmur1, xsc1a = ln_finish(ps_sum1, ps_sq1, x1t, TLOC, "ln1")
    cc_x1 = dram.tile([D + 1, TLOC], BF, tag="cc_x1")
    x1ng = dram.tile([G, D + 1, TLOC], BF, tag="x1ng")
    sync.dma_start(out=_ap(cc_x1, 0, [[TLOC, P], [P * TLOC, 8], [1, TLOC]]),
                   in_=xsc1a)
    sync.dma_start(out=cc_x1[D:D + 1, :], in_=negmur1)
    gps.collective_compute("AllGather", ALU.bypass, replica_groups=RG,
                           ins=[cc_x1.opt()], outs=[x1ng.opt()])
    vt2 = make_vT(kv2_sb, "vt2")

    if KMODE < 6:
        ctx.close()
        return

    # =================== block1 (y = cross-attn queries) ===================
    st1 = (D + 1) * TLOC
    mv1a = sb.tile([P, 8, S], BF, tag="agin", bufs=1)
    for r_ in range(G):
        sync.dma_start(out=mv1a[:, :, r_ * TLOC:(r_ + 1) * TLOC],
                      in_=_ap(x1ng.tensor, r_ * st1,
                              [[TLOC, P], [P * TLOC, 8], [1, TLOC]]))
    mv1 = [mv1a[:, k, :] for k in range(8)]
    negmur1f = sb.tile([1, S], BF, tag="negmur", bufs=4)
    sync.dma_start(out=negmur1f, in_=_ap(x1ng.tensor, D * TLOC,
                                         [[1, 1], [st1, G], [1, TLOC]]))
    cc_y = dram.tile([G, D, TLOC], BF, tag="cc_y")
    yg = dram.tile([D, TLOC], BF, tag="yg")

    def emit1(m, b2t, ps):
        o = sb.tile([P, S], BF, tag="qkvband", bufs=4)
        if m % 2:
            vec.tensor_scalar_add(o, ps, b2t[:, m:m + 1])
        else:
            act.add(o, ps, b2t[:, m:m + 1])
        sync.dma_start(
            out=_ap(cc_y, m * P * TLOC, [[TLOC, P], [D * TLOC, G], [1, TLOC]]),
            in_=o)

    ffn(1, W1, mv1, negmur1f, emit1)
    gps.collective_compute("ReduceScatter", ALU.add, replica_groups=RG,
                           ins=[cc_y.opt()], outs=[yg.opt()])

    if KMODE < 7:
        ctx.close()
        return

    # =================== cross-attention ===================
    yh_all = sb.tile([DIM, H, TLOC], BF, tag="yh", bufs=1)
    sync.dma_start(out=yh_all, in_=_ap(yg, 0,
                                       [[TLOC, DIM], [DIM * TLOC, H], [1, TLOC]]))

    def s_src1(j):
        s_ps = psum.tile([P, S], FP32, tag="ps_attn", bufs=2)
        for hh in range(2):
            h = 2 * j + hh
            pe.matmul(s_ps[hh * 64:(hh + 1) * 64, :], yh_all[:, h, :],
                      kv2_sb[:, h, 0, :], start=True, stop=True)
        return s_ps

    # x2 = x1 + a1 -> output + LN3 -> AllGather; LN3 stats ride along
    x2a = sb.tile([P, 8, TLOC], FP32, tag="x2", bufs=1)
    ps_sum3 = psum.tile([P, TLOC], FP32, tag="ps_stat", bufs=2)
    ps_sq3 = psum.tile([P, TLOC], FP32, tag="ps_stat", bufs=2)
    sq3 = sb.tile([P, 8, TLOC], BF, tag="lnsq", bufs=1)

    def consume1(j, a_ps):
        vec.tensor_add(x2a[:, j, :], x1a[:, j, :], a_ps)
        pe.matmul(ps_sum3[0:1, :], ones_colf, x2a[:, j, :],
                  start=(j == 0), stop=(j == 7))
        vec.tensor_mul(sq3[:, j, :], x2a[:, j, :], x2a[:, j, :])
        pe.matmul(ps_sq3[0:1, :], ones_col, sq3[:, j, :],
                  start=(j == 0), stop=(j == 7))

    softmax_av(s_src1, vt2, consume1)

    x2t = [x2a[:, k, :] for k in range(8)]
    sync.dma_start(out=_ap(io["x2T_out"], 0, [[TLOC, P], [P * TLOC, 8], [1, TLOC]]),
                   in_=x2a)
    xsc3, negmur3, xsc3a = ln_finish(ps_sum3, ps_sq3, x2t, TLOC, "ln3")
    cc_x3 = dram.tile([D + 1, TLOC], BF, tag="cc_x3")
    x3ng = dram.tile([G, D + 1, TLOC], BF, tag="x3ng")
    sync.dma_start(out=_ap(cc_x3, 0, [[TLOC, P], [P * TLOC, 8], [1, TLOC]]),
                   in_=xsc3a)
    sync.dma_start(out=cc_x3[D:D + 1, :], in_=negmur3)
    gps.collective_compute("AllGather", ALU.bypass, replica_groups=RG,
                           ins=[cc_x3.opt()], outs=[x3ng.opt()])

    if KMODE < 8:
        ctx.close()
        return

    # =================== block3 (partials; host sums ranks) ===================
    mv3a = sb.tile([P, 8, S], BF, tag="agin", bufs=1)
    for r_ in range(G):
        sync.dma_start(out=mv3a[:, :, r_ * TLOC:(r_ + 1) * TLOC],
                      in_=_ap(x3ng.tensor, r_ * st1,
                              [[TLOC, P], [P * TLOC, 8], [1, TLOC]]))
    mv3 = [mv3a[:, k, :] for k in range(8)]
    negmur3f = sb.tile([1, S], BF, tag="negmur", bufs=4)
    sync.dma_start(out=negmur3f, in_=_ap(x3ng.tensor, D * TLOC,
                                         [[1, 1], [st1, G], [1, TLOC]]))

    def emit3(m, b2t, ps):
        o = sb.tile([P, S], BF, tag="o3", bufs=2)
        vec.tensor_scalar_add(o, ps, b2t[:, m:m + 1])
        act.dma_start(out=io["o3T_out"].ap()[m * P:(m + 1) * P, :], in_=o)

    ffn(3, W3, mv3, negmur3f, emit3)
    ctx.close()


# ------------------------------------------------------------------- runner
def kernel(**inputs) -> np.ndarray:
    if "nc" not in _CACHE:
        _CACHE["nc"] = _build_nc()
    nc = _CACHE["nc"]
    in_maps = _prep_in_maps(inputs)
    res = run_bass_kernel_spmd(nc, in_maps, core_ids=list(range(8)))
    out = np.zeros((B, S, D), np.float32)
    for g in range(B):
        x2T = np.concatenate(
            [np.asarray(res.results[g * G + s]["x2T_out"]) for s in range(G)], axis=1)
        o3T = np.sum(
            [np.asarray(res.results[g * G + s]["o3T_out"]).astype(np.float32)
             for s in range(G)], axis=0)
        out[g] = (x2T + o3T).T
    return out
